# revision 2
# baseline (speedup 1.0000x reference)
"""Trainium2 Bass kernel for nn_Discriminator_455266534113 (relational GCN discriminator).

Data-parallel across 8 NeuronCores: batch 512 -> 64 per core. All weights replicated.

Layout strategy (host-side prep, device does zero transposes):
  - A [512,128,128,5] -> per core AT [64, n=128, r*128+m]  (adjacency pre-transposed so the
    contraction index n lands on SBUF partitions; contiguous DMA)
  - X -> per core XG [16 groups, 33, 4*128]  (features transposed, 4 batch elems side by side,
    row 32 = ones so bias folds into the matmul contraction)
  - Per-relation weights concatenated along free dim with a bias row appended:
    WL1 [33, 5*64], WL2 [65, 5*128]; feature-branch WF1 [33,64], WF2 [65,128].

Device dataflow per batch elem b (all "transposed" layout, features on partitions):
  h1 = relu(xT_aug.T @ WL1)            [n=128, 320]   one matmul
  agg1T = sum_r h1_r-as-lhsT @ adjT_r  [64, 128]      5 accumulating matmuls
  x1T = tanh(agg1T + relu(WF1.T @ xT)) [64+1, 128]
  (same again for layer 2 -> x2T [128, 128])
  iT = sigmoid(WI.T @ x2T + bi); jT = tanh(WJ.T @ iT + bj)   batched 4-wide (512 free)
  g = sum_n iT*jT -> tanh -> head via W1, W2 -> out [1, 64] per core
"""

import sys
from contextlib import ExitStack

import numpy as np

if "/opt/trn_rl_repo" not in sys.path:
    sys.path.insert(0, "/opt/trn_rl_repo")

B, N, R, F = 512, 128, 5, 32
H1, H2 = 64, 128
NCORES, BPC, G = 8, 64, 4
NG = BPC // G  # 16 groups per core

_F32 = None  # set lazily (mybir import)


def _build_nc():
    import concourse.bass as bass
    import concourse.mybir as mybir
    import concourse.tile as tile
    from concourse import bacc

    f32 = mybir.dt.float32
    AF = mybir.ActivationFunctionType

    nc = bacc.Bacc("TRN2", target_bir_lowering=False, debug=False)

    AT = nc.dram_tensor("AT", [BPC, N, R * N], f32, kind="ExternalInput").ap()
    XG = nc.dram_tensor("XG", [NG, F + 1, G * N], f32, kind="ExternalInput").ap()
    WL1 = nc.dram_tensor("WL1", [F + 1, R * H1], f32, kind="ExternalInput").ap()
    WF1 = nc.dram_tensor("WF1", [F + 1, H1], f32, kind="ExternalInput").ap()
    WL2 = nc.dram_tensor("WL2", [H1 + 1, R * H2], f32, kind="ExternalInput").ap()
    WF2 = nc.dram_tensor("WF2", [H1 + 1, H2], f32, kind="ExternalInput").ap()
    WI = nc.dram_tensor("WI", [H2, 128], f32, kind="ExternalInput").ap()
    BI = nc.dram_tensor("BI", [128, 1], f32, kind="ExternalInput").ap()
    WJ = nc.dram_tensor("WJ", [128, 128], f32, kind="ExternalInput").ap()
    BJ = nc.dram_tensor("BJ", [128, 1], f32, kind="ExternalInput").ap()
    W1 = nc.dram_tensor("W1", [128, 128], f32, kind="ExternalInput").ap()
    B1 = nc.dram_tensor("B1", [128, 1], f32, kind="ExternalInput").ap()
    W2 = nc.dram_tensor("W2", [128, 1], f32, kind="ExternalInput").ap()
    B2 = nc.dram_tensor("B2", [1, 1], f32, kind="ExternalInput").ap()
    OUT = nc.dram_tensor("OUT", [1, BPC], f32, kind="ExternalOutput").ap()

    with tile.TileContext(nc) as tc, ExitStack() as ctx:
        const = ctx.enter_context(tc.tile_pool(name="const", bufs=1))
        a_pool = ctx.enter_context(tc.tile_pool(name="a_pool", bufs=8))
        xg_pool = ctx.enter_context(tc.tile_pool(name="xg_pool", bufs=2))
        h1_pool = ctx.enter_context(tc.tile_pool(name="h1_pool", bufs=2))
        h2_pool = ctx.enter_context(tc.tile_pool(name="h2_pool", bufs=2))
        f1_pool = ctx.enter_context(tc.tile_pool(name="f1_pool", bufs=2))
        f2_pool = ctx.enter_context(tc.tile_pool(name="f2_pool", bufs=2))
        x1_pool = ctx.enter_context(tc.tile_pool(name="x1_pool", bufs=2))
        x2_pool = ctx.enter_context(tc.tile_pool(name="x2_pool", bufs=2))
        i_pool = ctx.enter_context(tc.tile_pool(name="i_pool", bufs=2))
        j_pool = ctx.enter_context(tc.tile_pool(name="j_pool", bufs=2))
        p_pool = ctx.enter_context(tc.tile_pool(name="p_pool", bufs=2))

        ps_h1 = ctx.enter_context(tc.tile_pool(name="ps_h1", bufs=2, space="PSUM"))
        ps_a1 = ctx.enter_context(tc.tile_pool(name="ps_a1", bufs=1, space="PSUM"))
        ps_h2 = ctx.enter_context(tc.tile_pool(name="ps_h2", bufs=1, space="PSUM"))
        ps_a2 = ctx.enter_context(tc.tile_pool(name="ps_a2", bufs=1, space="PSUM"))
        ps_g = ctx.enter_context(tc.tile_pool(name="ps_g", bufs=2, space="PSUM"))

        # --- constants to SBUF ---
        def cload(ap, shape, tag):
            t = const.tile(shape, f32, tag=tag)
            nc.sync.dma_start(t[:], ap)
            return t

        wl1 = cload(WL1, [F + 1, R * H1], "wl1")
        wf1 = cload(WF1, [F + 1, H1], "wf1")
        wl2 = cload(WL2, [H1 + 1, R * H2], "wl2")
        wf2 = cload(WF2, [H1 + 1, H2], "wf2")
        wi = cload(WI, [H2, 128], "wi")
        bi = cload(BI, [128, 1], "bi")
        wj = cload(WJ, [128, 128], "wj")
        bj = cload(BJ, [128, 1], "bj")
        w1 = cload(W1, [128, 128], "w1")
        b1 = cload(B1, [128, 1], "b1")
        w2 = cload(W2, [128, 1], "w2")
        b2 = cload(B2, [1, 1], "b2")
        g_raw = const.tile([128, BPC], f32, tag="g_raw")

        for g in range(NG):
            xg = xg_pool.tile([F + 1, G * N], f32, tag="xg")
            nc.sync.dma_start(xg[:], XG[g])
            ats = []
            for j in range(G):
                at = a_pool.tile([N, R * N], f32, tag="at")
                nc.sync.dma_start(at[:], AT[G * g + j])
                ats.append(at)

            # feat1T = relu(WF1.T @ xg)  [64, 512]
            f1p = ps_g.tile([H1, G * N], f32, tag="psg")
            nc.tensor.matmul(f1p[:], lhsT=wf1[:], rhs=xg[:], start=True, stop=True)
            f1s = f1_pool.tile([H1, G * N], f32, tag="f1s")
            nc.scalar.activation(f1s[:], f1p[:], AF.Relu)

            x1g = x1_pool.tile([H1 + 1, G * N], f32, tag="x1g")
            nc.gpsimd.memset(x1g[H1:H1 + 1, :], 1.0)

            # ---- layer 1, per batch elem ----
            for j in range(G):
                js = slice(j * N, (j + 1) * N)
                h1p = ps_h1.tile([N, R * H1], f32, tag="h1p")
                nc.tensor.matmul(h1p[:], lhsT=xg[:, js], rhs=wl1[:], start=True, stop=True)
                h1s = h1_pool.tile([N, R * H1], f32, tag="h1s")
                nc.vector.tensor_scalar_max(h1s[:], h1p[:], 0.0)
                a1p = ps_a1.tile([H1, N], f32, tag="a1p")
                for r in range(R):
                    nc.tensor.matmul(
                        a1p[:],
                        lhsT=h1s[:, r * H1:(r + 1) * H1],
                        rhs=ats[j][:, r * N:(r + 1) * N],
                        start=(r == 0),
                        stop=(r == R - 1),
                    )
                nc.vector.tensor_add(x1g[0:H1, js], a1p[:], f1s[:, js])
            nc.scalar.activation(x1g[0:H1, :], x1g[0:H1, :], AF.Tanh)

            # feat2T = relu(WF2.T @ x1g)  [128, 512]
            f2p = ps_g.tile([H2, G * N], f32, tag="psg")
            nc.tensor.matmul(f2p[:], lhsT=wf2[:], rhs=x1g[:], start=True, stop=True)
            f2s = f2_pool.tile([H2, G * N], f32, tag="f2s")
            nc.scalar.activation(f2s[:], f2p[:], AF.Relu)

            x2g = x2_pool.tile([H2, G * N], f32, tag="x2g")

            # ---- layer 2, per batch elem ----
            for j in range(G):
                js = slice(j * N, (j + 1) * N)
                h2p = ps_h2.tile([N, R * H2], f32, tag="h2p")
                nc.tensor.matmul(h2p[:, 0:512], lhsT=x1g[:, js], rhs=wl2[:, 0:512], start=True, stop=True)
                nc.tensor.matmul(h2p[:, 512:640], lhsT=x1g[:, js], rhs=wl2[:, 512:640], start=True, stop=True)
                h2s = h2_pool.tile([N, R * H2], f32, tag="h2s")
                nc.vector.tensor_scalar_max(h2s[:, 0:320], h2p[:, 0:320], 0.0)
                nc.scalar.activation(h2s[:, 320:640], h2p[:, 320:640], AF.Relu)
                a2p = ps_a2.tile([H2, N], f32, tag="a2p")
                for r in range(R):
                    nc.tensor.matmul(
                        a2p[:],
                        lhsT=h2s[:, r * H2:(r + 1) * H2],
                        rhs=ats[j][:, r * N:(r + 1) * N],
                        start=(r == 0),
                        stop=(r == R - 1),
                    )
                nc.vector.tensor_add(x2g[:, js], a2p[:], f2s[:, js])
            nc.scalar.activation(x2g[:], x2g[:], AF.Tanh)

            # ---- gated aggregation, batched 4-wide ----
            ip = ps_g.tile([128, G * N], f32, tag="psg")
            nc.tensor.matmul(ip[:], lhsT=wi[:], rhs=x2g[:], start=True, stop=True)
            is_ = i_pool.tile([128, G * N], f32, tag="is")
            nc.scalar.activation(is_[:], ip[:], AF.Sigmoid, bias=bi[:])
            jp = ps_g.tile([128, G * N], f32, tag="psg")
            nc.tensor.matmul(jp[:], lhsT=wj[:], rhs=is_[:], start=True, stop=True)
            js_t = j_pool.tile([128, G * N], f32, tag="js")
            nc.scalar.activation(js_t[:], jp[:], AF.Tanh, bias=bj[:])
            prod = p_pool.tile([128, G * N], f32, tag="prod")
            nc.vector.tensor_mul(prod[:], is_[:], js_t[:])
            nc.vector.tensor_reduce(
                g_raw[:, G * g:G * (g + 1)],
                prod[:].rearrange("p (j n) -> p j n", n=N),
                axis=mybir.AxisListType.X,
                op=mybir.AluOpType.add,
            )

        # ---- head, once per core ----
        gt = const.tile([128, BPC], f32, tag="gt")
        nc.scalar.activation(gt[:], g_raw[:], AF.Tanh)
        hp = ps_g.tile([128, BPC], f32, tag="psg")
        nc.tensor.matmul(hp[:], lhsT=w1[:], rhs=gt[:], start=True, stop=True)
        hs = const.tile([128, BPC], f32, tag="hs")
        nc.scalar.activation(hs[:], hp[:], AF.Tanh, bias=b1[:])
        op = ps_g.tile([1, BPC], f32, tag="psg")
        nc.tensor.matmul(op[:], lhsT=w2[:], rhs=hs[:], start=True, stop=True)
        os_ = const.tile([1, BPC], f32, tag="os")
        nc.scalar.activation(os_[:], op[:], AF.Tanh, bias=b2[:])
        nc.sync.dma_start(OUT, os_[:])

    nc.compile()
    return nc


_NC_CACHE = None


def _get_nc():
    global _NC_CACHE
    if _NC_CACHE is None:
        _NC_CACHE = _build_nc()
    return _NC_CACHE


def host_prep(inputs):
    A = np.ascontiguousarray(np.asarray(inputs["A"], dtype=np.float32))
    X = np.asarray(inputs["X"], dtype=np.float32)
    f32 = np.float32

    def arr(name):
        return np.ascontiguousarray(np.asarray(inputs[name], dtype=f32))

    Wl1, bl1 = arr("Wl1"), arr("bl1")
    Wf1, bf1 = arr("Wf1"), arr("bf1")
    Wl2, bl2 = arr("Wl2"), arr("bl2")
    Wf2, bf2 = arr("Wf2"), arr("bf2")

    W = {
        "WL1": np.ascontiguousarray(
            np.concatenate([Wl1.transpose(1, 0, 2).reshape(F, R * H1), bl1.reshape(1, R * H1)], 0)
        ),
        "WF1": np.ascontiguousarray(np.concatenate([Wf1, bf1[None]], 0)),
        "WL2": np.ascontiguousarray(
            np.concatenate([Wl2.transpose(1, 0, 2).reshape(H1, R * H2), bl2.reshape(1, R * H2)], 0)
        ),
        "WF2": np.ascontiguousarray(np.concatenate([Wf2, bf2[None]], 0)),
        "WI": arr("Wi"),
        "BI": np.ascontiguousarray(arr("bi").reshape(128, 1)),
        "WJ": arr("Wj"),
        "BJ": np.ascontiguousarray(arr("bj").reshape(128, 1)),
        "W1": arr("W1"),
        "B1": np.ascontiguousarray(arr("b1").reshape(128, 1)),
        "W2": arr("W2"),
        "B2": np.ascontiguousarray(arr("b2").reshape(1, 1)),
    }

    in_maps = []
    for c in range(NCORES):
        bs = slice(c * BPC, (c + 1) * BPC)
        AT = np.ascontiguousarray(A[bs].transpose(0, 2, 3, 1)).reshape(BPC, N, R * N)
        Xt = (
            X[bs]
            .transpose(0, 2, 1)
            .reshape(NG, G, F, N)
            .transpose(0, 2, 1, 3)
            .reshape(NG, F, G * N)
        )
        XG = np.ascontiguousarray(
            np.concatenate([Xt, np.ones((NG, 1, G * N), f32)], 1)
        )
        in_maps.append({"AT": AT, "XG": XG, **W})
    return in_maps


def kernel(**inputs) -> np.ndarray:
    from concourse.bass_utils import run_bass_kernel_spmd

    in_maps = host_prep(inputs)
    nc = _get_nc()
    res = run_bass_kernel_spmd(nc, in_maps, core_ids=list(range(NCORES)))
    out = np.concatenate([r["OUT"].reshape(BPC) for r in res.results])
    return out.reshape(B, 1).astype(np.float32)


# revision 5
# speedup vs baseline: 7.6970x; 7.6970x over previous
"""Trainium2 Bass kernel for nn_Discriminator_455266534113 (relational GCN discriminator).

Data-parallel across 8 NeuronCores: batch 512 -> 64 per core. All weights replicated.

Layout strategy (host-side prep, device does zero transposes):
  - A [512,128,128,5] -> per core AT [64, n=128, r*128+m]  (adjacency pre-transposed so the
    contraction index n lands on SBUF partitions; contiguous DMA)
  - X -> per core XG [16 groups, 33, 4*128]  (features transposed, 4 batch elems side by side,
    row 32 = ones so bias folds into the matmul contraction)
  - Per-relation weights concatenated along free dim with a bias row appended:
    WL1 [33, 5*64], WL2 [65, 5*128]; feature-branch WF1 [33,64], WF2 [65,128].

Device dataflow per batch elem b (all "transposed" layout, features on partitions):
  h1 = relu(xT_aug.T @ WL1)            [n=128, 320]   one matmul
  agg1T = sum_r h1_r-as-lhsT @ adjT_r  [64, 128]      5 accumulating matmuls
  x1T = tanh(agg1T + relu(WF1.T @ xT)) [64+1, 128]
  (same again for layer 2 -> x2T [128, 128])
  iT = sigmoid(WI.T @ x2T + bi); jT = tanh(WJ.T @ iT + bj)   batched 4-wide (512 free)
  g = sum_n iT*jT -> tanh -> head via W1, W2 -> out [1, 64] per core
"""

import sys
from contextlib import ExitStack

import numpy as np

if "/opt/trn_rl_repo" not in sys.path:
    sys.path.insert(0, "/opt/trn_rl_repo")

B, N, R, F = 512, 128, 5, 32
H1, H2 = 64, 128
NCORES, BPC, G = 8, 64, 4
NG = BPC // G  # 16 groups per core

_F32 = None  # set lazily (mybir import)


def _build_nc(rep: int = 1):
    import concourse.bass as bass
    import concourse.mybir as mybir
    import concourse.tile as tile
    from concourse import bacc

    f32 = mybir.dt.float32
    AF = mybir.ActivationFunctionType

    nc = bacc.Bacc("TRN2", target_bir_lowering=False, debug=False)

    AT = nc.dram_tensor("AT", [BPC, N, R * N], f32, kind="ExternalInput").ap()
    XG = nc.dram_tensor("XG", [NG, F + 1, G * N], f32, kind="ExternalInput").ap()
    WL1 = nc.dram_tensor("WL1", [F + 1, R * H1], f32, kind="ExternalInput").ap()
    WF1 = nc.dram_tensor("WF1", [F + 1, H1], f32, kind="ExternalInput").ap()
    WL2 = nc.dram_tensor("WL2", [H1 + 1, R * H2], f32, kind="ExternalInput").ap()
    WF2 = nc.dram_tensor("WF2", [H1 + 1, H2], f32, kind="ExternalInput").ap()
    WI = nc.dram_tensor("WI", [H2, 128], f32, kind="ExternalInput").ap()
    BI = nc.dram_tensor("BI", [128, 1], f32, kind="ExternalInput").ap()
    WJ = nc.dram_tensor("WJ", [128, 128], f32, kind="ExternalInput").ap()
    BJ = nc.dram_tensor("BJ", [128, 1], f32, kind="ExternalInput").ap()
    W1 = nc.dram_tensor("W1", [128, 128], f32, kind="ExternalInput").ap()
    B1 = nc.dram_tensor("B1", [128, 1], f32, kind="ExternalInput").ap()
    W2 = nc.dram_tensor("W2", [128, 1], f32, kind="ExternalInput").ap()
    B2 = nc.dram_tensor("B2", [1, 1], f32, kind="ExternalInput").ap()
    OUT = nc.dram_tensor("OUT", [1, BPC], f32, kind="ExternalOutput").ap()

    with tile.TileContext(nc) as tc, ExitStack() as ctx:
        const = ctx.enter_context(tc.tile_pool(name="const", bufs=1))
        a_pool = ctx.enter_context(tc.tile_pool(name="a_pool", bufs=8))
        xg_pool = ctx.enter_context(tc.tile_pool(name="xg_pool", bufs=2))
        h1_pool = ctx.enter_context(tc.tile_pool(name="h1_pool", bufs=2))
        h2_pool = ctx.enter_context(tc.tile_pool(name="h2_pool", bufs=2))
        f1_pool = ctx.enter_context(tc.tile_pool(name="f1_pool", bufs=2))
        f2_pool = ctx.enter_context(tc.tile_pool(name="f2_pool", bufs=2))
        x1_pool = ctx.enter_context(tc.tile_pool(name="x1_pool", bufs=2))
        x2_pool = ctx.enter_context(tc.tile_pool(name="x2_pool", bufs=2))
        i_pool = ctx.enter_context(tc.tile_pool(name="i_pool", bufs=2))
        j_pool = ctx.enter_context(tc.tile_pool(name="j_pool", bufs=2))
        p_pool = ctx.enter_context(tc.tile_pool(name="p_pool", bufs=2))

        ps_h1 = ctx.enter_context(tc.tile_pool(name="ps_h1", bufs=2, space="PSUM"))
        ps_a1 = ctx.enter_context(tc.tile_pool(name="ps_a1", bufs=1, space="PSUM"))
        ps_h2 = ctx.enter_context(tc.tile_pool(name="ps_h2", bufs=1, space="PSUM"))
        ps_a2 = ctx.enter_context(tc.tile_pool(name="ps_a2", bufs=1, space="PSUM"))
        ps_g = ctx.enter_context(tc.tile_pool(name="ps_g", bufs=2, space="PSUM"))

        # --- constants to SBUF ---
        def cload(ap, shape, tag):
            t = const.tile(shape, f32, tag=tag)
            nc.sync.dma_start(t[:], ap)
            return t

        wl1 = cload(WL1, [F + 1, R * H1], "wl1")
        wf1 = cload(WF1, [F + 1, H1], "wf1")
        wl2 = cload(WL2, [H1 + 1, R * H2], "wl2")
        wf2 = cload(WF2, [H1 + 1, H2], "wf2")
        wi = cload(WI, [H2, 128], "wi")
        bi = cload(BI, [128, 1], "bi")
        wj = cload(WJ, [128, 128], "wj")
        bj = cload(BJ, [128, 1], "bj")
        w1 = cload(W1, [128, 128], "w1")
        b1 = cload(B1, [128, 1], "b1")
        w2 = cload(W2, [128, 1], "w2")
        b2 = cload(B2, [1, 1], "b2")
        g_raw = const.tile([128, BPC], f32, tag="g_raw")

        for g in range(NG * rep):
            g = g % NG
            xg = xg_pool.tile([F + 1, G * N], f32, tag="xg")
            nc.sync.dma_start(xg[:], XG[g])
            ats = []
            for j in range(G):
                at = a_pool.tile([N, R * N], f32, tag="at")
                nc.sync.dma_start(at[:], AT[G * g + j])
                ats.append(at)

            # feat1T = relu(WF1.T @ xg)  [64, 512]
            f1p = ps_g.tile([H1, G * N], f32, tag="psg")
            nc.tensor.matmul(f1p[:], lhsT=wf1[:], rhs=xg[:], start=True, stop=True)
            f1s = f1_pool.tile([H1, G * N], f32, tag="f1s")
            nc.scalar.activation(f1s[:], f1p[:], AF.Relu)

            x1g = x1_pool.tile([H1 + 1, G * N], f32, tag="x1g")
            nc.gpsimd.memset(x1g[H1:H1 + 1, :], 1.0)

            # ---- layer 1, per batch elem ----
            for j in range(G):
                js = slice(j * N, (j + 1) * N)
                h1p = ps_h1.tile([N, R * H1], f32, tag="h1p")
                nc.tensor.matmul(h1p[:], lhsT=xg[:, js], rhs=wl1[:], start=True, stop=True)
                h1s = h1_pool.tile([N, R * H1], f32, tag="h1s")
                nc.vector.tensor_scalar_max(h1s[:], h1p[:], 0.0)
                a1p = ps_a1.tile([H1, N], f32, tag="a1p")
                for r in range(R):
                    nc.tensor.matmul(
                        a1p[:],
                        lhsT=h1s[:, r * H1:(r + 1) * H1],
                        rhs=ats[j][:, r * N:(r + 1) * N],
                        start=(r == 0),
                        stop=(r == R - 1),
                    )
                nc.vector.tensor_add(x1g[0:H1, js], a1p[:], f1s[:, js])
            nc.scalar.activation(x1g[0:H1, :], x1g[0:H1, :], AF.Tanh)

            # feat2T = relu(WF2.T @ x1g)  [128, 512]
            f2p = ps_g.tile([H2, G * N], f32, tag="psg")
            nc.tensor.matmul(f2p[:], lhsT=wf2[:], rhs=x1g[:], start=True, stop=True)
            f2s = f2_pool.tile([H2, G * N], f32, tag="f2s")
            nc.scalar.activation(f2s[:], f2p[:], AF.Relu)

            x2g = x2_pool.tile([H2, G * N], f32, tag="x2g")

            # ---- layer 2, per batch elem ----
            for j in range(G):
                js = slice(j * N, (j + 1) * N)
                h2p = ps_h2.tile([N, R * H2], f32, tag="h2p")
                nc.tensor.matmul(h2p[:, 0:512], lhsT=x1g[:, js], rhs=wl2[:, 0:512], start=True, stop=True)
                nc.tensor.matmul(h2p[:, 512:640], lhsT=x1g[:, js], rhs=wl2[:, 512:640], start=True, stop=True)
                h2s = h2_pool.tile([N, R * H2], f32, tag="h2s")
                nc.vector.tensor_scalar_max(h2s[:, 0:320], h2p[:, 0:320], 0.0)
                nc.scalar.activation(h2s[:, 320:640], h2p[:, 320:640], AF.Relu)
                a2p = ps_a2.tile([H2, N], f32, tag="a2p")
                for r in range(R):
                    nc.tensor.matmul(
                        a2p[:],
                        lhsT=h2s[:, r * H2:(r + 1) * H2],
                        rhs=ats[j][:, r * N:(r + 1) * N],
                        start=(r == 0),
                        stop=(r == R - 1),
                    )
                nc.vector.tensor_add(x2g[:, js], a2p[:], f2s[:, js])
            nc.scalar.activation(x2g[:], x2g[:], AF.Tanh)

            # ---- gated aggregation, batched 4-wide ----
            ip = ps_g.tile([128, G * N], f32, tag="psg")
            nc.tensor.matmul(ip[:], lhsT=wi[:], rhs=x2g[:], start=True, stop=True)
            is_ = i_pool.tile([128, G * N], f32, tag="is")
            nc.scalar.activation(is_[:], ip[:], AF.Sigmoid, bias=bi[:])
            jp = ps_g.tile([128, G * N], f32, tag="psg")
            nc.tensor.matmul(jp[:], lhsT=wj[:], rhs=is_[:], start=True, stop=True)
            js_t = j_pool.tile([128, G * N], f32, tag="js")
            nc.scalar.activation(js_t[:], jp[:], AF.Tanh, bias=bj[:])
            prod = p_pool.tile([128, G * N], f32, tag="prod")
            nc.vector.tensor_mul(prod[:], is_[:], js_t[:])
            nc.vector.tensor_reduce(
                g_raw[:, G * g:G * (g + 1)],
                prod[:].rearrange("p (j n) -> p j n", n=N),
                axis=mybir.AxisListType.X,
                op=mybir.AluOpType.add,
            )

        # ---- head, once per core ----
        gt = const.tile([128, BPC], f32, tag="gt")
        nc.scalar.activation(gt[:], g_raw[:], AF.Tanh)
        hp = ps_g.tile([128, BPC], f32, tag="psg")
        nc.tensor.matmul(hp[:], lhsT=w1[:], rhs=gt[:], start=True, stop=True)
        hs = const.tile([128, BPC], f32, tag="hs")
        nc.scalar.activation(hs[:], hp[:], AF.Tanh, bias=b1[:])
        op = ps_g.tile([1, BPC], f32, tag="psg")
        nc.tensor.matmul(op[:], lhsT=w2[:], rhs=hs[:], start=True, stop=True)
        os_ = const.tile([1, BPC], f32, tag="os")
        nc.scalar.activation(os_[:], op[:], AF.Tanh, bias=b2[:])
        nc.sync.dma_start(OUT, os_[:])

    nc.compile()
    return nc


_NC_CACHE = {}


def _get_nc(rep: int = 1):
    if rep not in _NC_CACHE:
        _NC_CACHE[rep] = _build_nc(rep)
    return _NC_CACHE[rep]


def host_prep(inputs):
    A = np.ascontiguousarray(np.asarray(inputs["A"], dtype=np.float32))
    X = np.asarray(inputs["X"], dtype=np.float32)
    f32 = np.float32

    def arr(name):
        return np.ascontiguousarray(np.asarray(inputs[name], dtype=f32))

    Wl1, bl1 = arr("Wl1"), arr("bl1")
    Wf1, bf1 = arr("Wf1"), arr("bf1")
    Wl2, bl2 = arr("Wl2"), arr("bl2")
    Wf2, bf2 = arr("Wf2"), arr("bf2")

    W = {
        "WL1": np.ascontiguousarray(
            np.concatenate([Wl1.transpose(1, 0, 2).reshape(F, R * H1), bl1.reshape(1, R * H1)], 0)
        ),
        "WF1": np.ascontiguousarray(np.concatenate([Wf1, bf1[None]], 0)),
        "WL2": np.ascontiguousarray(
            np.concatenate([Wl2.transpose(1, 0, 2).reshape(H1, R * H2), bl2.reshape(1, R * H2)], 0)
        ),
        "WF2": np.ascontiguousarray(np.concatenate([Wf2, bf2[None]], 0)),
        "WI": arr("Wi"),
        "BI": np.ascontiguousarray(arr("bi").reshape(128, 1)),
        "WJ": arr("Wj"),
        "BJ": np.ascontiguousarray(arr("bj").reshape(128, 1)),
        "W1": arr("W1"),
        "B1": np.ascontiguousarray(arr("b1").reshape(128, 1)),
        "W2": arr("W2"),
        "B2": np.ascontiguousarray(arr("b2").reshape(1, 1)),
    }

    in_maps = []
    for c in range(NCORES):
        bs = slice(c * BPC, (c + 1) * BPC)
        AT = np.ascontiguousarray(A[bs].transpose(0, 2, 3, 1)).reshape(BPC, N, R * N)
        Xt = (
            X[bs]
            .transpose(0, 2, 1)
            .reshape(NG, G, F, N)
            .transpose(0, 2, 1, 3)
            .reshape(NG, F, G * N)
        )
        XG = np.ascontiguousarray(
            np.concatenate([Xt, np.ones((NG, 1, G * N), f32)], 1)
        )
        in_maps.append({"AT": AT, "XG": XG, **W})
    return in_maps


def kernel(**inputs) -> np.ndarray:
    from concourse.bass_utils import run_bass_kernel_spmd

    in_maps = host_prep(inputs)
    nc = _get_nc()
    res = run_bass_kernel_spmd(nc, in_maps, core_ids=list(range(NCORES)))
    out = np.concatenate([r["OUT"].reshape(BPC) for r in res.results])
    return out.reshape(B, 1).astype(np.float32)


# revision 15
# speedup vs baseline: 85.7867x; 11.1454x over previous
"""Trainium2 Bass kernel for nn_Discriminator_455266534113 (relational GCN discriminator).

Data-parallel across 8 NeuronCores: batch 512 -> 64 per core. All weights replicated.

Layout strategy (host-side prep, device does zero transposes):
  - A [512,128,128,5] -> per core AT [64, n=128, r*128+m] in bf16 (adjacency pre-transposed
    so the contraction index n lands on SBUF partitions; contiguous DMA, half the HBM traffic)
  - X -> per core XG [16 groups, 33, 4*128] f32 (features transposed, 4 batch elems side by
    side, row 32 = ones so bias folds into the matmul contraction)
  - Per-relation weights concatenated along free dim with a bias row appended:
    WL1 [33, 5*64], WL2 [65, 5*128]; feature-branch WF1 [33,64], WF2 [65,128].

Precision strategy (validated numerically: final rel err ~1e-4 vs f32 reference):
  - Adjacency (uniform[0,1)) and post-relu h tensors in bf16 -> the 40 small (Nf=128)
    aggregation matmuls run at 1 cycle/row with fast weight load.
  - All wide matmuls (Nf>=256) use float32r operands (full fp32 bits, fast PE mode).
  - PSUM accumulation is always fp32; layer 1 is saturated (z1 > 46) so bf16 noise vanishes.

Per batch elem b (all "transposed" layout, features on partitions):
  h1 = relu(xT_aug.T @ WL1)              [n=128, 320]   one f32r matmul, bf16 eviction
  aggT1 slice = sum_r h1_r.T @ adjT_r  (+ relu(WF1.T @ xT) injected via identity matmul)
  x1T = tanh(aggT1)  -- one ACT op per group of 4, read straight from PSUM
  (same for layer 2 -> x2T), then gated aggregation batched 4-wide:
  iT = sigmoid(WI.T@x2T+bi); jT = tanh(WJ.T@iT+bj); g_raw = sum_n iT*jT (Pool mul + DVE reduce)
  head: tanh(g) -> W1 -> tanh -> W2 -> tanh -> out [1, 64] per core
"""

import sys
from contextlib import ExitStack

import numpy as np

if "/opt/trn_rl_repo" not in sys.path:
    sys.path.insert(0, "/opt/trn_rl_repo")

B, N, R, F = 512, 128, 5, 32
H1, H2 = 64, 128
NCORES, BPC, G = 8, 64, 4
NG = BPC // G  # 16 groups per core


def _build_nc(rep: int = 1, wide_dt: str = "layers"):
    import concourse.bass as bass
    import concourse.mybir as mybir
    import concourse.tile as tile
    from concourse import bacc
    from concourse.masks import make_identity

    f32 = mybir.dt.float32
    bf16 = mybir.dt.bfloat16
    import os
    _mode = os.environ.get("WIDE_DT", wide_dt)  # all | layers | gated | none
    if _mode == "f32r":
        _mode = "all"
    if _mode == "f32":
        _mode = "none"
    dt_l = mybir.dt.float32r if _mode in ("all", "layers") else mybir.dt.float32
    dt_g = mybir.dt.float32r if _mode in ("all", "gated") else mybir.dt.float32
    AF = mybir.ActivationFunctionType

    nc = bacc.Bacc("TRN2", target_bir_lowering=False, debug=False)

    AT = nc.dram_tensor("AT", [BPC, N, R * N], bf16, kind="ExternalInput").ap()
    XG = nc.dram_tensor("XG", [NG, F + 1, G * N], dt_l, kind="ExternalInput").ap()
    WL1 = nc.dram_tensor("WL1", [F + 1, R * H1], dt_l, kind="ExternalInput").ap()
    WF1 = nc.dram_tensor("WF1", [F + 1, H1], dt_l, kind="ExternalInput").ap()
    WL2 = nc.dram_tensor("WL2", [H1 + 1, R * H2], dt_l, kind="ExternalInput").ap()
    WF2 = nc.dram_tensor("WF2", [H1 + 1, H2], dt_l, kind="ExternalInput").ap()
    WI = nc.dram_tensor("WI", [H2, 128], dt_g, kind="ExternalInput").ap()
    BI = nc.dram_tensor("BI", [128, 1], f32, kind="ExternalInput").ap()
    WJ = nc.dram_tensor("WJ", [128, 128], dt_g, kind="ExternalInput").ap()
    BJ = nc.dram_tensor("BJ", [128, 1], f32, kind="ExternalInput").ap()
    W1 = nc.dram_tensor("W1", [128, 128], dt_g, kind="ExternalInput").ap()
    B1 = nc.dram_tensor("B1", [128, 1], f32, kind="ExternalInput").ap()
    W2 = nc.dram_tensor("W2", [128, 1], dt_g, kind="ExternalInput").ap()
    B2 = nc.dram_tensor("B2", [1, 1], f32, kind="ExternalInput").ap()
    OUT = nc.dram_tensor("OUT", [1, BPC], f32, kind="ExternalOutput").ap()

    with tile.TileContext(nc) as tc, ExitStack() as ctx:
        const = ctx.enter_context(tc.tile_pool(name="const", bufs=1))
        a_pool = ctx.enter_context(tc.tile_pool(name="a_pool", bufs=3))
        xg_pool = ctx.enter_context(tc.tile_pool(name="xg_pool", bufs=2))
        h1_pool = ctx.enter_context(tc.tile_pool(name="h1_pool", bufs=4))
        h2_pool = ctx.enter_context(tc.tile_pool(name="h2_pool", bufs=4))
        f1_pool = ctx.enter_context(tc.tile_pool(name="f1_pool", bufs=3))
        f2_pool = ctx.enter_context(tc.tile_pool(name="f2_pool", bufs=3))
        x1_pool = ctx.enter_context(tc.tile_pool(name="x1_pool", bufs=3))
        x2_pool = ctx.enter_context(tc.tile_pool(name="x2_pool", bufs=3))
        i_pool = ctx.enter_context(tc.tile_pool(name="i_pool", bufs=2))
        j_pool = ctx.enter_context(tc.tile_pool(name="j_pool", bufs=2))
        p_pool = ctx.enter_context(tc.tile_pool(name="p_pool", bufs=2))

        # PSUM: 8 banks total -> 3 + 1 + 2 + 2
        ps_h = ctx.enter_context(tc.tile_pool(name="ps_h", bufs=3, space="PSUM"))
        ps_a1 = ctx.enter_context(tc.tile_pool(name="ps_a1", bufs=1, space="PSUM"))
        ps_a2 = ctx.enter_context(tc.tile_pool(name="ps_a2", bufs=2, space="PSUM"))
        ps_g = ctx.enter_context(tc.tile_pool(name="ps_g", bufs=2, space="PSUM"))

        def cload(ap, shape, tag, dt=f32):
            t = const.tile(shape, dt, tag=tag)
            nc.sync.dma_start(t[:], ap)
            return t

        wl1 = cload(WL1, [F + 1, R * H1], "wl1", dt_l)
        wf1 = cload(WF1, [F + 1, H1], "wf1", dt_l)
        wl2 = cload(WL2, [H1 + 1, R * H2], "wl2", dt_l)
        wf2 = cload(WF2, [H1 + 1, H2], "wf2", dt_l)
        wi = cload(WI, [H2, 128], "wi", dt_g)
        bi = cload(BI, [128, 1], "bi")
        wj = cload(WJ, [128, 128], "wj", dt_g)
        bj = cload(BJ, [128, 1], "bj")
        w1 = cload(W1, [128, 128], "w1", dt_g)
        b1 = cload(B1, [128, 1], "b1")
        w2 = cload(W2, [128, 1], "w2", dt_g)
        b2 = cload(B2, [1, 1], "b2")
        g_raw = const.tile([128, BPC], f32, tag="g_raw")
        i64 = const.tile([H1, H1], bf16, tag="i64")
        make_identity(nc, i64[:])
        i128 = const.tile([H2, H2], bf16, tag="i128")
        make_identity(nc, i128[:])

        def emit_tail(g, a2p):
            """Gated aggregation for group g — emitted one group late so its
            serial ACT/PE ping-pong overlaps the next group's dense work."""
            x2g = x2_pool.tile([H2, G * N], dt_g, tag="x2g")
            nc.scalar.activation(x2g[:], a2p[:], AF.Tanh)
            ip = ps_g.tile([128, G * N], f32, tag="psg")
            nc.tensor.matmul(ip[:], lhsT=wi[:], rhs=x2g[:], start=True, stop=True)
            is_ = i_pool.tile([128, G * N], dt_g, tag="is")
            nc.scalar.activation(is_[:], ip[:], AF.Sigmoid, bias=bi[:])
            jp = ps_g.tile([128, G * N], f32, tag="psg")
            nc.tensor.matmul(jp[:], lhsT=wj[:], rhs=is_[:], start=True, stop=True)
            js_t = j_pool.tile([128, G * N], f32, tag="js")
            nc.scalar.activation(js_t[:], jp[:], AF.Tanh, bias=bj[:])
            prod = p_pool.tile([128, G * N], f32, tag="prod")
            nc.gpsimd.tensor_mul(prod[:], is_[:].bitcast(f32), js_t[:])
            nc.vector.tensor_reduce(
                g_raw[:, G * g:G * (g + 1)],
                prod[:].rearrange("p (j n) -> p j n", n=N),
                axis=mybir.AxisListType.X,
                op=mybir.AluOpType.add,
            )

        pending = None  # (g, a2p) of the previous group
        for g in range(NG * rep):
            g = g % NG
            xg = xg_pool.tile([F + 1, G * N], dt_l, tag="xg")
            nc.sync.dma_start(xg[:], XG[g])
            # all 4 adjacency tiles in one DMA (HWDGE descriptor cost is per dma_start)
            at_g = a_pool.tile([N, G * R * N], bf16, tag="at")
            nc.sync.dma_start(
                at_g[:].rearrange("n (j m) -> n j m", m=R * N),
                AT[G * g:G * (g + 1)].rearrange("j n m -> n j m"),
            )
            ats = [at_g[:, j * R * N:(j + 1) * R * N] for j in range(G)]

            # feat1T = relu(WF1.T @ xg) -> bf16  [64, 512]
            f1p = ps_g.tile([H1, G * N], f32, tag="psg")
            nc.tensor.matmul(f1p[:], lhsT=wf1[:], rhs=xg[:], start=True, stop=True)
            f1s = f1_pool.tile([H1, G * N], bf16, tag="f1s")
            nc.scalar.activation(f1s[:], f1p[:], AF.Relu)

            x1g = x1_pool.tile([H1 + 1, G * N], dt_l, tag="x1g")
            nc.gpsimd.memset(x1g[H1:H1 + 1, :].bitcast(f32), 1.0)

            # ---- layer 1 ----
            a1p = ps_a1.tile([H1, G * N], f32, tag="a1p")
            for j in range(G):
                js = slice(j * N, (j + 1) * N)
                h1p = ps_h.tile([N, R * H1], f32, tag="ph")
                nc.tensor.matmul(h1p[:], lhsT=xg[:, js], rhs=wl1[:], start=True, stop=True)
                h1s = h1_pool.tile([N, R * H1], bf16, tag="h1s")
                if j < 2:  # rebalance: DVE is the busiest engine, ACT has headroom
                    nc.scalar.activation(h1s[:], h1p[:], AF.Relu)
                else:
                    nc.vector.tensor_scalar_max(h1s[:], h1p[:], 0.0)
                for rr in range(R):
                    nc.tensor.matmul(
                        a1p[:, js],
                        lhsT=h1s[:, rr * H1:(rr + 1) * H1],
                        rhs=ats[j][:, rr * N:(rr + 1) * N],
                        start=(rr == 0),
                        stop=False,
                    )
                nc.tensor.matmul(a1p[:, js], lhsT=i64[:], rhs=f1s[:, js], start=False, stop=True)
            nc.scalar.activation(x1g[0:H1, :], a1p[:], AF.Tanh)

            # feat2T = relu(WF2.T @ x1g) -> bf16  [128, 512]
            f2p = ps_g.tile([H2, G * N], f32, tag="psg")
            nc.tensor.matmul(f2p[:], lhsT=wf2[:], rhs=x1g[:], start=True, stop=True)
            f2s = f2_pool.tile([H2, G * N], bf16, tag="f2s")
            nc.scalar.activation(f2s[:], f2p[:], AF.Relu)

            # ---- layer 2 ----
            a2p = ps_a2.tile([H2, G * N], f32, tag="a2p")
            for j in range(G):
                js = slice(j * N, (j + 1) * N)
                h2pa = ps_h.tile([N, 320], f32, tag="ph")
                nc.tensor.matmul(h2pa[:], lhsT=x1g[:, js], rhs=wl2[:, 0:320], start=True, stop=True)
                h2pb = ps_h.tile([N, 320], f32, tag="ph")
                nc.tensor.matmul(h2pb[:], lhsT=x1g[:, js], rhs=wl2[:, 320:640], start=True, stop=True)
                h2s = h2_pool.tile([N, R * H2], bf16, tag="h2s")
                nc.vector.tensor_scalar_max(h2s[:, 0:320], h2pa[:], 0.0)
                nc.vector.tensor_scalar_max(h2s[:, 320:640], h2pb[:], 0.0)
                for rr in range(R):
                    nc.tensor.matmul(
                        a2p[:, js],
                        lhsT=h2s[:, rr * H2:(rr + 1) * H2],
                        rhs=ats[j][:, rr * N:(rr + 1) * N],
                        start=(rr == 0),
                        stop=False,
                    )
                nc.tensor.matmul(a2p[:, js], lhsT=i128[:], rhs=f2s[:, js], start=False, stop=True)

            if pending is not None:
                emit_tail(*pending)
            pending = (g, a2p)
        emit_tail(*pending)

        # ---- head, once per core ----
        gt = const.tile([128, BPC], dt_g, tag="gt")
        nc.scalar.activation(gt[:], g_raw[:], AF.Tanh)
        hp = ps_g.tile([128, BPC], f32, tag="psg")
        nc.tensor.matmul(hp[:], lhsT=w1[:], rhs=gt[:], start=True, stop=True)
        hs = const.tile([128, BPC], dt_g, tag="hs")
        nc.scalar.activation(hs[:], hp[:], AF.Tanh, bias=b1[:])
        op = ps_g.tile([1, BPC], f32, tag="psg")
        nc.tensor.matmul(op[:], lhsT=w2[:], rhs=hs[:], start=True, stop=True)
        os_ = const.tile([1, BPC], f32, tag="os")
        nc.scalar.activation(os_[:], op[:], AF.Tanh, bias=b2[:])
        nc.sync.dma_start(OUT, os_[:])

    nc.compile()
    return nc


_NC_CACHE = {}


def _get_nc(rep: int = 1):
    if rep not in _NC_CACHE:
        _NC_CACHE[rep] = _build_nc(rep)
    return _NC_CACHE[rep]


def host_prep(inputs):
    import ml_dtypes

    A = np.asarray(inputs["A"], dtype=np.float32)
    X = np.asarray(inputs["X"], dtype=np.float32)
    f32 = np.float32

    def arr(name):
        return np.ascontiguousarray(np.asarray(inputs[name], dtype=f32))

    Wl1, bl1 = arr("Wl1"), arr("bl1")
    Wf1, bf1 = arr("Wf1"), arr("bf1")
    Wl2, bl2 = arr("Wl2"), arr("bl2")
    Wf2, bf2 = arr("Wf2"), arr("bf2")

    W = {
        "WL1": np.ascontiguousarray(
            np.concatenate([Wl1.transpose(1, 0, 2).reshape(F, R * H1), bl1.reshape(1, R * H1)], 0)
        ),
        "WF1": np.ascontiguousarray(np.concatenate([Wf1, bf1[None]], 0)),
        "WL2": np.ascontiguousarray(
            np.concatenate([Wl2.transpose(1, 0, 2).reshape(H1, R * H2), bl2.reshape(1, R * H2)], 0)
        ),
        "WF2": np.ascontiguousarray(np.concatenate([Wf2, bf2[None]], 0)),
        "WI": arr("Wi"),
        "BI": np.ascontiguousarray(arr("bi").reshape(128, 1)),
        "WJ": arr("Wj"),
        "BJ": np.ascontiguousarray(arr("bj").reshape(128, 1)),
        "W1": arr("W1"),
        "B1": np.ascontiguousarray(arr("b1").reshape(128, 1)),
        "W2": arr("W2"),
        "B2": np.ascontiguousarray(arr("b2").reshape(1, 1)),
    }

    in_maps = []
    for c in range(NCORES):
        bs = slice(c * BPC, (c + 1) * BPC)
        AT = np.ascontiguousarray(
            A[bs].transpose(0, 2, 3, 1).reshape(BPC, N, R * N).astype(ml_dtypes.bfloat16)
        )
        Xt = (
            X[bs]
            .transpose(0, 2, 1)
            .reshape(NG, G, F, N)
            .transpose(0, 2, 1, 3)
            .reshape(NG, F, G * N)
        )
        XGa = np.ascontiguousarray(
            np.concatenate([Xt, np.ones((NG, 1, G * N), f32)], 1)
        )
        in_maps.append({"AT": AT, "XG": XGa, **W})
    return in_maps


def kernel(**inputs) -> np.ndarray:
    from concourse.bass_utils import run_bass_kernel_spmd

    in_maps = host_prep(inputs)
    nc = _get_nc()
    res = run_bass_kernel_spmd(nc, in_maps, core_ids=list(range(NCORES)))
    out = np.concatenate([r["OUT"].reshape(BPC) for r in res.results])
    return out.reshape(B, 1).astype(np.float32)
